# revision 13
# baseline (speedup 1.0000x reference)
"""BERT self-attention Bass kernel for 8 Trainium2 NeuronCores.

Problem: hidden_states [2, 2048, 768], 12 heads x 64 dim, fp32.

Sharding (zero-communication): core c in 0..7 handles batch b = c//4 and
head-group g = c%4 (3 heads).  Host pre-lays per-core inputs (fp16):
  - hsT  [768, 2048]  hidden[b].T
  - wqk  [128, 6*384] QK weights, partition-major: group h of 128 cols =
                      [q_h | k_h] per kc chunk (softmax 1/8 folded into Wq)
  - wv   [128, 6*192] V weight columns for the 3 heads, partition-major
  - bqk  [128, 3]     per-group bias [bq_h | bk_h] (fp32)
  - maskr [128, 16]   attention_mask[b] column-tiled per key tile

Device pipeline per core (fp16 matmuls, fp32 psum):
  1. hsT streams in 12 column-half pieces spread over both HWDGE rings
     and the SWDGE ring, ordered so the s/t 0:1024 half lands first; the
     ACT ring carries almost nothing so the scalar engine's sequencer
     (which generates its ring's DMA descriptors) is free to start the
     exp stream early.  A dummy activation right at kernel start pulls
     the ~1.3us exp table load off the critical path.
  2. Phase A: G0 = [q0|k0] projection per column half, kc-major, paced
     by the piece DMAs with warm-up fillers bridging gaps (HAM clock).
     q lands in psum rows 0:64, k in rows 64:128; DVE copies them to the
     same rows of qd0/ktp0 and an SBUF->SBUF DMA on the sync ring
     mirrors each into the other 64-partition half (engines cannot shift
     partitions) so both concurrent scores strips have their own copy.
  3. V[t, d] computed directly per key tile: 6 accumulating matmuls with
     the hsT chunk as the stationary operand (t-major, no PE transposes).
     vaug[:, tt, h, :] = exp(mask_t) * [V_h | 1] fp16.  The 16 V units
     are woven into head 0's early attention slack.
  4. Attention, head-sequential, per (head, s-block of 512): 16 key
     tiles as 8 exp groups of 2.  The two scores matmuls of a group run
     CONCURRENTLY in disjoint 64-row strips of the PE array
     (tile_position row tiling), halving the scores stream cost.  exp
     over [128, 1024] psum on ACT (unnormalized; scores are O(6) by
     construction, exp(mask) folded into V).  PV matmuls (M=65: V plus
     a ones column that yields the softmax denominator) lag the exp
     stream by several groups so the PE absorbs V/G1/G2 work in its
     slack without stalling ACT, the bottleneck engine (~107us of exp).
  5. G1/G2 projections + copies (DVE) run inside earlier heads' slack.
Host: divide rows 0..63 by row 64, transpose, add bv, interleave heads.
"""

import os

import numpy as np

import concourse.mybir as mybir
import concourse.tile as tile
from concourse import bacc
from concourse.bass_utils import run_bass_kernel_spmd

F32 = mybir.dt.float32
F16 = mybir.dt.float16

B = 2
S = 2048
HID = 768
NH = 12          # total heads
D = 64           # head dim
NHL = 3          # heads per core
DG = NHL * D     # 192 cols of each W per core
KC = HID // 128  # 6 contraction chunks
NT = S // 128    # 16 key tiles
SBW = 512        # s-block width
NSB = S // SBW   # 4 s-blocks
NGRP = 8         # exp groups of 2 key tiles per (head, s-block)

LAST_EXEC_TIME_NS = None

_CACHED_NC = None


def _build_nc():
    nc = bacc.Bacc("TRN2", target_bir_lowering=False, debug=False, num_devices=8)

    hsT_d = nc.dram_tensor("hsT", [HID, S], F16, kind="ExternalInput")
    wqk_d = nc.dram_tensor("wqk", [128, KC * 512], F16, kind="ExternalInput")
    wv_d = nc.dram_tensor("wv", [128, KC * DG], F16, kind="ExternalInput")
    bqk_d = nc.dram_tensor("bqk", [128, NHL + 1], F32, kind="ExternalInput")
    maskr_d = nc.dram_tensor("maskr", [128, NT], F32, kind="ExternalInput")
    out_d = nc.dram_tensor("ctxa", [NHL, D + 1, S], F32, kind="ExternalOutput")

    with tile.TileContext(nc) as tc:
        with (
            tc.tile_pool(name="const", bufs=1) as cp,
            tc.tile_pool(name="qk", bufs=1) as qp,
            tc.tile_pool(name="probs", bufs=7) as pp,
            tc.tile_pool(name="oc", bufs=3) as op,
            tc.tile_pool(name="ps_sc", bufs=3, space="PSUM") as ps_sc,
            tc.tile_pool(name="ps_cx", bufs=1, space="PSUM") as ps_cx,
            tc.tile_pool(name="ps_v", bufs=1, space="PSUM") as ps_v,
        ):
            # dummy activation: loads the exp spline tables immediately
            dummy = cp.tile([128, 8], F32, tag="dummy")
            nc.vector.memset(dummy[:], 0.0)
            nc.scalar.activation(
                dummy[:], dummy[:], mybir.ActivationFunctionType.Exp
            )

            wfsrc = cp.tile([128, 16], F16, tag="wfsrc")
            nc.gpsimd.memset(wfsrc[:], 1.0)

            # --- input DMAs ---
            # hsT as 4 column-quarter DMAs on the sync ring alone: one
            # InstDMACopy fans across all 16 SDMA engines, so a lone big
            # transfer runs near the ~358 GB/s HBM rate and quarter q
            # (all kc chunks of s/t columns q*512:(q+1)*512) completes in
            # arrival order for quarter-paced phase A.
            # quarter-major: quarter q is a contiguous byte range per
            # partition, so the interval-based subtile deps are precise
            # and quarter-paced compute starts as each quarter lands
            hs_all = cp.tile([128, 4, KC, SBW], F16, tag="hsT")

            def hsap(kc, c0, cw):
                q, o = divmod(c0, SBW)
                assert o + cw <= SBW
                return hs_all[:, q, kc, o : o + cw]

            wqk_sb = cp.tile([128, KC, 512], F16, tag="wqk")
            wv_sb = cp.tile([128, KC, DG], F16, tag="wv")
            bqk_sb = cp.tile([128, NHL + 1], F32, tag="bqk")
            maskr_sb = cp.tile([128, NT], F32, tag="maskr")

            hsT_pm = hsT_d.ap().rearrange("(kc p) (nq s) -> p nq kc s", p=128, s=SBW)
            nc.scalar.dma_start(
                wqk_sb[:].rearrange("p kc n -> p (kc n)"), wqk_d.ap()
            )
            nc.sync.dma_start(bqk_sb[:], bqk_d.ap())
            nc.sync.dma_start(maskr_sb[:], maskr_d.ap())
            for q in range(4):
                nc.sync.dma_start(
                    hs_all[:, q, :, :], hsT_pm[:, q, :, :]
                )
                if q == 1:
                    nc.sync.dma_start(
                        wv_sb[:].rearrange("p kc n -> p (kc n)"), wv_d.ap()
                    )

            # em[t] = exp(mask_t), folded into V_aug below
            em = cp.tile([128, NT], F32, tag="em")
            nc.scalar.activation(
                em[:], maskr_sb[:], mybir.ActivationFunctionType.Exp
            )

            # q rows 0:64 mirrored to 64:128 (k rows 64:128 to 0:64) by
            # SBUF->SBUF DMA so both scores strips have operands.
            qd = [
                qp.tile([128, S], F16, tag=f"qd{h}", name=f"qd{h}")
                for h in range(NHL)
            ]
            ktp = [
                qp.tile([128, S], F16, tag=f"ktp{h}", name=f"ktp{h}")
                for h in range(NHL)
            ]
            # vaug[:, tt, h, 0:64] = em_t * V_h[t, :], col 64 = em_t
            vaug = qp.tile([128, NT, NHL, D + 1], F16, tag="vaug")

            # --- PE warm-up fillers (no DMA dependency) ---
            wf_tiles = iter(
                [
                    ps_v.tile([128, 512], F32, tag="v", name=f"wf{i}")
                    for i in range(24)
                ]
            )

            def warm(n):
                for _ in range(n):
                    wf = next(wf_tiles, None)
                    if wf is None:
                        return
                    nc.tensor.matmul(
                        wf[:16, :16],
                        wfsrc[:],
                        wfsrc[:],
                        start=True,
                        stop=True,
                        skip_group_check=True,
                    )

            def qk_copy(h, acc, c0, cw):
                """psum acc [128, cw] -> qd[h] rows 0:64 / ktp[h] rows
                64:128, cols c0:c0+cw (partition-aligned, DVE)."""
                nc.vector.tensor_scalar_add(
                    qd[h][0:D, c0 : c0 + cw],
                    acc[0:D, :],
                    bqk_sb[0:D, h : h + 1],
                )
                nc.vector.tensor_scalar_add(
                    ktp[h][D:128, c0 : c0 + cw],
                    acc[D:128, :],
                    bqk_sb[D:128, h : h + 1],
                )

            def qk_dup(h, c0, cw):
                # mirror halves via SBUF->SBUF DMA on the (otherwise
                # idle) SWDGE ring; a blocked wait here stalls only the
                # gpsimd queue, never a compute engine's sequencer
                nc.gpsimd.dma_start(
                    qd[h][D:128, c0 : c0 + cw], qd[h][0:D, c0 : c0 + cw]
                )
                nc.gpsimd.dma_start(
                    ktp[h][0:D, c0 : c0 + cw], ktp[h][D:128, c0 : c0 + cw]
                )

            # --- Phase A: G0 = [q0|k0] per column quarter, kc-major.
            # Quarters 0-1 also run the flipped group [k0|q0] so their
            # mirror halves come straight from the PE (no dup latency on
            # the critical path); quarters 2-3 use dup DMAs whose
            # latency hides behind early attention.
            warm(24)
            for q in range(4):
                acc = ps_sc.tile([128, 512], F32, tag="sc", name=f"g0a{q}")
                accf = (
                    ps_sc.tile([128, 512], F32, tag="sc", name=f"g0f{q}")
                    if q < 2
                    else None
                )
                for kc in range(KC):
                    nc.tensor.matmul(
                        acc[:, :],
                        wqk_sb[:, kc, 0:128],
                        hsap(kc, q * SBW, SBW),
                        start=(kc == 0),
                        stop=(kc == KC - 1),
                    )
                    if accf is not None:
                        nc.tensor.matmul(
                            accf[:, :],
                            wqk_sb[:, kc, 128:256],
                            hsap(kc, q * SBW, SBW),
                            start=(kc == 0),
                            stop=(kc == KC - 1),
                        )
                qk_copy(0, acc, q * SBW, SBW)
                if accf is not None:
                    nc.vector.tensor_scalar_add(
                        ktp[0][0:D, q * SBW : (q + 1) * SBW],
                        accf[0:D, :],
                        bqk_sb[0:D, NHL : NHL + 1],
                    )
                    nc.vector.tensor_scalar_add(
                        qd[0][D:128, q * SBW : (q + 1) * SBW],
                        accf[D:128, :],
                        bqk_sb[D:128, NHL : NHL + 1],
                    )
                else:
                    qk_dup(0, q * SBW, SBW)
            # ones/em column of vaug for all tiles, one strided copy
            nc.vector.tensor_copy(
                vaug[:, :, :, D : D + 1],
                em[:, :]
                .rearrange("p (t o u) -> p t o u", o=1, u=1)
                .broadcast_to([128, NT, NHL, 1]),
            )

            # --- interleave units (run inside attention PE slack) ---
            def v_unit(tt):
                def run(tt=tt):
                    vps = ps_v.tile([128, DG], F32, tag="v", name="vps")
                    for kc in range(KC):
                        nc.tensor.matmul(
                            vps[:, :],
                            hsap(kc, tt * 128, 128),
                            wv_sb[:, kc, :],
                            start=(kc == 0),
                            stop=(kc == KC - 1),
                        )
                    nc.vector.tensor_scalar_mul(
                        vaug[:, tt, :, 0:D],
                        vps[:].rearrange("p (h d) -> p h d", d=D),
                        em[:, tt : tt + 1],
                    )

                return run

            def g_unit(h, a):
                """G_h projection for s-quarter a + copies (DVE) + dup."""

                def run(h=h, a=a):
                    acc = ps_v.tile([128, 512], F32, tag="v", name="gacc")
                    for kc in range(KC):
                        nc.tensor.matmul(
                            acc[:, :],
                            wqk_sb[:, kc, (h + 1) * 128 : (h + 2) * 128],
                            hsap(kc, a * SBW, SBW),
                            start=(kc == 0),
                            stop=(kc == KC - 1),
                        )
                    qk_copy(h, acc, a * SBW, SBW)
                    qk_dup(h, a * SBW, SBW)

                return run

            jit = [v_unit(tt) for tt in range(NT)]
            jit += [g_unit(1, a) for a in range(NSB)]
            jit += [g_unit(2, a) for a in range(NSB)]
            # cumulative jit targets per (head, s-block) 0..11
            jit_goal = [12, 16, 18, 20, 21, 22, 23, 24, 24, 24, 24, 24]
            jit_pos = [0]

            def run_jit(goal):
                while jit_pos[0] < min(goal, len(jit)):
                    jit[jit_pos[0]]()
                    jit_pos[0] += 1

            # --- attention ---
            pending = []  # (h, ctx, t0, pr, last)

            def flush_pending(n):
                for _ in range(max(0, min(n, len(pending)))):
                    (h, ctx, t0, pr, last) = pending.pop(0)
                    if h == 0:
                        # vaug tiles this PV reads must be emitted first
                        run_jit(t0 + 2)
                    for i in range(2):
                        tt = t0 + i
                        nc.tensor.matmul(
                            ctx[: D + 1, :],
                            vaug[:, tt, h, :],
                            pr[:, i * SBW : (i + 1) * SBW],
                            start=(tt == 0),
                            stop=(tt == NT - 1),
                        )
                    if last:
                        h_, s0_ = last
                        oc = op.tile([128, SBW], F32, tag="oc", name="oc")
                        nc.vector.tensor_copy(oc[: D + 1, :], ctx[: D + 1, :])
                        nc.sync.dma_start(
                            out_d.ap()[h_, :, s0_ : s0_ + SBW],
                            oc[: D + 1, :],
                        )

            for hsb in range(NHL * NSB):
                h, sb = divmod(hsb, NSB)
                s0 = sb * SBW
                ctx = ps_cx.tile([128, SBW], F32, tag="cx", name="ctx")
                goal_prev = jit_goal[hsb - 1] if hsb else 0
                goal_cur = jit_goal[hsb]
                for g in range(NGRP):
                    sc = ps_sc.tile([128, 1024], F32, tag="sc", name="sc")
                    # two concurrent K=64 strip matmuls (rows 0:64 / 64:128)
                    for half in range(2):
                        tt = 2 * g + half
                        r = slice(0, D) if half == 0 else slice(D, 128)
                        nc.tensor.matmul(
                            sc[:, half * SBW : (half + 1) * SBW],
                            ktp[h][r, tt * 128 : (tt + 1) * 128],
                            qd[h][r, s0 : s0 + SBW],
                            start=True,
                            stop=True,
                        )
                    pr = pp.tile([128, 1024], F16, tag="pr", name="pr")
                    nc.scalar.activation(
                        pr[:], sc[:], mybir.ActivationFunctionType.Exp
                    )
                    run_jit(goal_prev + (goal_cur - goal_prev) * (g + 1) // NGRP)
                    last = (h, s0) if g == NGRP - 1 else None
                    pending.append((h, ctx, 2 * g, pr, last))
                    maxlag = 5 if hsb < 2 else (2 if hsb == NHL * NSB - 1 else 3)
                    flush_pending(len(pending) - maxlag)
            flush_pending(len(pending))

    nc.compile()
    return nc


def _get_nc():
    global _CACHED_NC
    if _CACHED_NC is None:
        _CACHED_NC = _build_nc()
    return _CACHED_NC


def kernel(
    hidden_states, attention_mask, Wq, bq, Wk, bk, Wv, bv
) -> np.ndarray:
    global LAST_EXEC_TIME_NS
    hidden_states = np.asarray(hidden_states, dtype=np.float32)
    attention_mask = np.asarray(attention_mask, dtype=np.float32)
    Wq = np.asarray(Wq, dtype=np.float32)
    Wk = np.asarray(Wk, dtype=np.float32)
    Wv = np.asarray(Wv, dtype=np.float32)
    bq = np.asarray(bq, dtype=np.float32)
    bk = np.asarray(bk, dtype=np.float32)
    bv = np.asarray(bv, dtype=np.float32)

    scale = 1.0 / np.sqrt(np.float32(D))

    in_maps = []
    for c in range(8):
        b, g = divmod(c, 4)
        cols = slice(g * DG, (g + 1) * DG)
        wq = Wq[:, cols] * scale
        wk = Wk[:, cols]
        wv = Wv[:, cols]
        bq_, bk_ = bq[cols] * scale, bk[cols]
        wqk = np.zeros((HID, 512), dtype=np.float32)
        bqk = np.zeros((128, NHL + 1), dtype=np.float32)
        for h in range(NHL):
            c0 = h * 128 if h == 0 else (h + 1) * 128
            wqk[:, c0 : c0 + D] = wq[:, h * D : (h + 1) * D]
            wqk[:, c0 + D : c0 + 128] = wk[:, h * D : (h + 1) * D]
            bqk[0:D, h] = bq_[h * D : (h + 1) * D]
            bqk[D:128, h] = bk_[h * D : (h + 1) * D]
        # flipped group [k0|q0] at cols 128:256, bias col NHL = [bk0|bq0]
        wqk[:, 128 : 128 + D] = wk[:, 0:D]
        wqk[:, 128 + D : 256] = wq[:, 0:D]
        bqk[0:D, NHL] = bk_[0:D]
        bqk[D:128, NHL] = bq_[0:D]
        # partition-major relayouts: [hid, n] -> [128, kc*n]
        wqk_pm = np.ascontiguousarray(
            wqk.reshape(KC, 128, 512).transpose(1, 0, 2).reshape(128, -1)
        )
        wv_pm = np.ascontiguousarray(
            wv.reshape(KC, 128, DG).transpose(1, 0, 2).reshape(128, -1)
        )
        maskr = np.ascontiguousarray(
            attention_mask[b, 0, 0, :].reshape(NT, 128).T
        )
        in_maps.append(
            {
                "hsT": np.ascontiguousarray(hidden_states[b].T).astype(np.float16),
                "wqk": wqk_pm.astype(np.float16),
                "wv": wv_pm.astype(np.float16),
                "bqk": bqk,
                "maskr": maskr.astype(np.float32),
            }
        )

    nc = _get_nc()
    trace = bool(os.environ.get("BASS_KERNEL_TRACE"))
    res = run_bass_kernel_spmd(nc, in_maps, list(range(8)), trace=trace)
    LAST_EXEC_TIME_NS = res.exec_time_ns

    out = np.empty((B, S, HID), dtype=np.float32)
    for c in range(8):
        b, g = divmod(c, 4)
        ctxa = res.results[c]["ctxa"]  # [3, 65, 2048]
        for hl in range(NHL):
            ctx = ctxa[hl, :D, :] / ctxa[hl, D : D + 1, :]  # [64, 2048]
            out[b, :, g * DG + hl * D : g * DG + (hl + 1) * D] = (
                ctx.T + bv[g * DG + hl * D : g * DG + (hl + 1) * D]
            )
    return out


# revision 14
# speedup vs baseline: 1.2357x; 1.2357x over previous
"""BERT self-attention Bass kernel for 8 Trainium2 NeuronCores.

Problem: hidden_states [2, 2048, 768], 12 heads x 64 dim, fp32.

Sharding (zero-communication): core c in 0..7 handles batch b = c//4 and
head-group g = c%4 (3 heads).  Host pre-lays per-core inputs (fp16):
  - hsT  [768, 2048]  hidden[b].T
  - wqk  [128, 6*384] QK weights, partition-major: group h of 128 cols =
                      [q_h | k_h] per kc chunk (softmax 1/8 folded into Wq)
  - wv   [128, 6*192] V weight columns for the 3 heads, partition-major
  - bqk  [128, 3]     per-group bias [bq_h | bk_h] (fp32)
  - maskr [128, 16]   attention_mask[b] column-tiled per key tile

Device pipeline per core (fp16 matmuls, fp32 psum):
  1. hsT streams in 4 column-quarter DMAs on the sync HWDGE ring: one
     InstDMACopy fans across all 16 SDMA engines, so each big transfer
     runs near the HBM rate and quarters complete in arrival order.
     The ACT ring carries only wqk so the scalar engine's sequencer
     (which generates its ring's DMA descriptors) is free to start the
     exp stream early; a dummy activation right at kernel start pulls
     the ~1.3us exp table load off the critical path.
  2. Phase A: G0 = [q0|k0] projection per column quarter, kc-major,
     paced by the quarter DMAs with warm-up fillers bridging gaps (HAM
     clock).  q lands in psum rows 0:64, k in rows 64:128; DVE copies
     them to the same rows of qd0/ktp0 and an SBUF->SBUF DMA on the
     SWDGE ring mirrors each into the other 64-partition half (engines
     cannot shift partitions) so both concurrent scores strips have
     their own operand copy.
  3. V[t, d] computed directly per key tile: 6 accumulating matmuls with
     the hsT chunk as the stationary operand (t-major, no PE transposes).
     vaug[:, tt, h, :] = exp(mask_t) * [V_h | 1] fp16.  The 16 V units
     are woven into head 0's early attention slack.
  4. Attention, head-sequential, per (head, s-block of 512): 16 key
     tiles as 8 exp groups of 2.  The two scores matmuls of a group run
     CONCURRENTLY in disjoint 64-row strips of the PE array
     (tile_position row tiling), halving the scores stream cost.  exp
     over [128, 1024] psum on ACT (unnormalized; scores are O(6) by
     construction, exp(mask) folded into V).  PV matmuls (M=65: V plus
     a ones column that yields the softmax denominator) lag the exp
     stream by several groups so the PE absorbs V/G1/G2 work in its
     slack without stalling ACT, the bottleneck engine (~107us of exp).
  5. G1/G2 projections + copies (DVE) run inside earlier heads' slack.
Host: divide rows 0..63 by row 64, transpose, add bv, interleave heads.
"""

import os

import numpy as np

import concourse.mybir as mybir
import concourse.tile as tile
from concourse import bacc
from concourse.bass_utils import run_bass_kernel_spmd

F32 = mybir.dt.float32
F16 = mybir.dt.float16

B = 2
S = 2048
HID = 768
NH = 12          # total heads
D = 64           # head dim
NHL = 3          # heads per core
DG = NHL * D     # 192 cols of each W per core
KC = HID // 128  # 6 contraction chunks
NT = S // 128    # 16 key tiles
SBW = 512        # s-block width
NSB = S // SBW   # 4 s-blocks
NGRP = 8         # exp groups of 2 key tiles per (head, s-block)

LAST_EXEC_TIME_NS = None

_CACHED_NC = None


def _build_nc():
    nc = bacc.Bacc("TRN2", target_bir_lowering=False, debug=False, num_devices=8)

    hsT_d = nc.dram_tensor("hsT", [HID, S], F16, kind="ExternalInput")
    wqk_d = nc.dram_tensor("wqk", [128, KC * 384], F16, kind="ExternalInput")
    wv_d = nc.dram_tensor("wv", [128, KC * DG], F16, kind="ExternalInput")
    bqk_d = nc.dram_tensor("bqk", [128, NHL], F32, kind="ExternalInput")
    maskr_d = nc.dram_tensor("maskr", [128, NT], F32, kind="ExternalInput")
    out_d = nc.dram_tensor("ctxa", [NHL, D + 1, S], F32, kind="ExternalOutput")

    with tile.TileContext(nc) as tc:
        with (
            tc.tile_pool(name="const", bufs=1) as cp,
            tc.tile_pool(name="qk", bufs=1) as qp,
            tc.tile_pool(name="probs", bufs=7) as pp,
            tc.tile_pool(name="oc", bufs=3) as op,
            tc.tile_pool(name="ps_sc", bufs=3, space="PSUM") as ps_sc,
            tc.tile_pool(name="ps_cx", bufs=1, space="PSUM") as ps_cx,
            tc.tile_pool(name="ps_v", bufs=1, space="PSUM") as ps_v,
        ):
            # dummy activation: loads the exp spline tables immediately
            dummy = cp.tile([128, 8], F32, tag="dummy")
            nc.vector.memset(dummy[:], 0.0)
            nc.scalar.activation(
                dummy[:], dummy[:], mybir.ActivationFunctionType.Exp
            )

            wfsrc = cp.tile([128, 16], F16, tag="wfsrc")
            nc.gpsimd.memset(wfsrc[:], 1.0)

            # --- input DMAs ---
            # hsT as 4 column-quarter DMAs on the sync ring alone: one
            # InstDMACopy fans across all 16 SDMA engines, so a lone big
            # transfer runs near the ~358 GB/s HBM rate and quarter q
            # (all kc chunks of s/t columns q*512:(q+1)*512) completes in
            # arrival order for quarter-paced phase A.
            hs_all = cp.tile([128, KC, S], F16, tag="hsT")
            hs = [hs_all[:, kc, :] for kc in range(KC)]
            wqk_sb = cp.tile([128, KC, 384], F16, tag="wqk")
            wv_sb = cp.tile([128, KC, DG], F16, tag="wv")
            bqk_sb = cp.tile([128, NHL], F32, tag="bqk")
            maskr_sb = cp.tile([128, NT], F32, tag="maskr")

            hsT_pm = hsT_d.ap().rearrange("(kc p) s -> p kc s", p=128)
            nc.scalar.dma_start(
                wqk_sb[:].rearrange("p kc n -> p (kc n)"), wqk_d.ap()
            )
            nc.gpsimd.dma_start(
                wv_sb[:].rearrange("p kc n -> p (kc n)"), wv_d.ap()
            )
            nc.gpsimd.dma_start(bqk_sb[:], bqk_d.ap())
            nc.gpsimd.dma_start(maskr_sb[:], maskr_d.ap())
            for q in range(4):
                qs = slice(q * SBW, (q + 1) * SBW)
                nc.sync.dma_start(hs_all[:, :, qs], hsT_pm[:, :, qs])

            # em[t] = exp(mask_t), folded into V_aug below
            em = cp.tile([128, NT], F32, tag="em")
            nc.scalar.activation(
                em[:], maskr_sb[:], mybir.ActivationFunctionType.Exp
            )

            # q rows 0:64 mirrored to 64:128 (k rows 64:128 to 0:64) by
            # SBUF->SBUF DMA so both scores strips have operands.
            qd = [
                qp.tile([128, S], F16, tag=f"qd{h}", name=f"qd{h}")
                for h in range(NHL)
            ]
            ktp = [
                qp.tile([128, S], F16, tag=f"ktp{h}", name=f"ktp{h}")
                for h in range(NHL)
            ]
            # vaug[:, tt, h, 0:64] = em_t * V_h[t, :], col 64 = em_t
            vaug = qp.tile([128, NT, NHL, D + 1], F16, tag="vaug")

            # --- PE warm-up fillers (no DMA dependency) ---
            wf_tiles = iter(
                [
                    ps_v.tile([128, 512], F32, tag="v", name=f"wf{i}")
                    for i in range(24)
                ]
            )

            def warm(n):
                for _ in range(n):
                    wf = next(wf_tiles, None)
                    if wf is None:
                        return
                    nc.tensor.matmul(
                        wf[:16, :16],
                        wfsrc[:],
                        wfsrc[:],
                        start=True,
                        stop=True,
                        skip_group_check=True,
                    )

            def qk_copy(h, acc, c0, cw):
                """psum acc [128, cw] -> qd[h] rows 0:64 / ktp[h] rows
                64:128, cols c0:c0+cw (partition-aligned, DVE)."""
                nc.vector.tensor_scalar_add(
                    qd[h][0:D, c0 : c0 + cw],
                    acc[0:D, :],
                    bqk_sb[0:D, h : h + 1],
                )
                nc.vector.tensor_scalar_add(
                    ktp[h][D:128, c0 : c0 + cw],
                    acc[D:128, :],
                    bqk_sb[D:128, h : h + 1],
                )

            def qk_dup(h, c0, cw):
                # mirror halves via SBUF->SBUF DMA on the (otherwise
                # idle) SWDGE ring; a blocked wait here stalls only the
                # gpsimd queue, never a compute engine's sequencer
                nc.gpsimd.dma_start(
                    qd[h][D:128, c0 : c0 + cw], qd[h][0:D, c0 : c0 + cw]
                )
                nc.gpsimd.dma_start(
                    ktp[h][0:D, c0 : c0 + cw], ktp[h][D:128, c0 : c0 + cw]
                )

            # --- Phase A: G0 = [q0|k0] per column quarter, kc-major ---
            warm(6)
            NWARM = [0, 1, 1, 2, 2, 2]
            for q in range(4):
                acc = ps_sc.tile([128, 512], F32, tag="sc", name=f"g0a{q}")
                for kc in range(KC):
                    if q < 2:
                        warm(NWARM[kc])
                    nc.tensor.matmul(
                        acc[:, :],
                        wqk_sb[:, kc, 0:128],
                        hs[kc][:, q * SBW : (q + 1) * SBW],
                        start=(kc == 0),
                        stop=(kc == KC - 1),
                    )
                qk_copy(0, acc, q * SBW, SBW)
                qk_dup(0, q * SBW, SBW)
            # ones/em column of vaug for all tiles, one strided copy
            nc.vector.tensor_copy(
                vaug[:, :, :, D : D + 1],
                em[:, :]
                .rearrange("p (t o u) -> p t o u", o=1, u=1)
                .broadcast_to([128, NT, NHL, 1]),
            )

            # --- interleave units (run inside attention PE slack) ---
            def v_unit(tt):
                def run(tt=tt):
                    vps = ps_v.tile([128, DG], F32, tag="v", name="vps")
                    for kc in range(KC):
                        nc.tensor.matmul(
                            vps[:, :],
                            hs[kc][:, tt * 128 : (tt + 1) * 128],
                            wv_sb[:, kc, :],
                            start=(kc == 0),
                            stop=(kc == KC - 1),
                        )
                    nc.vector.tensor_scalar_mul(
                        vaug[:, tt, :, 0:D],
                        vps[:].rearrange("p (h d) -> p h d", d=D),
                        em[:, tt : tt + 1],
                    )

                return run

            def g_unit(h, a):
                """G_h projection for s-quarter a + copies (DVE) + dup."""

                def run(h=h, a=a):
                    acc = ps_v.tile([128, 512], F32, tag="v", name="gacc")
                    for kc in range(KC):
                        nc.tensor.matmul(
                            acc[:, :],
                            wqk_sb[:, kc, h * 128 : (h + 1) * 128],
                            hs[kc][:, a * SBW : (a + 1) * SBW],
                            start=(kc == 0),
                            stop=(kc == KC - 1),
                        )
                    qk_copy(h, acc, a * SBW, SBW)
                    qk_dup(h, a * SBW, SBW)

                return run

            jit = [v_unit(tt) for tt in range(NT)]
            jit += [g_unit(1, a) for a in range(NSB)]
            jit += [g_unit(2, a) for a in range(NSB)]
            # cumulative jit targets per (head, s-block) 0..11
            jit_goal = [12, 16, 18, 20, 21, 22, 23, 24, 24, 24, 24, 24]
            jit_pos = [0]

            def run_jit(goal):
                while jit_pos[0] < min(goal, len(jit)):
                    jit[jit_pos[0]]()
                    jit_pos[0] += 1

            # --- attention ---
            pending = []  # (h, ctx, t0, pr, last)

            def flush_pending(n):
                for _ in range(max(0, min(n, len(pending)))):
                    (h, ctx, t0, pr, last) = pending.pop(0)
                    if h == 0:
                        # vaug tiles this PV reads must be emitted first
                        run_jit(t0 + 2)
                    for i in range(2):
                        tt = t0 + i
                        nc.tensor.matmul(
                            ctx[: D + 1, :],
                            vaug[:, tt, h, :],
                            pr[:, i * SBW : (i + 1) * SBW],
                            start=(tt == 0),
                            stop=(tt == NT - 1),
                        )
                    if last:
                        h_, s0_ = last
                        oc = op.tile([128, SBW], F32, tag="oc", name="oc")
                        nc.vector.tensor_copy(oc[: D + 1, :], ctx[: D + 1, :])
                        nc.sync.dma_start(
                            out_d.ap()[h_, :, s0_ : s0_ + SBW],
                            oc[: D + 1, :],
                        )

            for hsb in range(NHL * NSB):
                h, sb = divmod(hsb, NSB)
                s0 = sb * SBW
                ctx = ps_cx.tile([128, SBW], F32, tag="cx", name="ctx")
                goal_prev = jit_goal[hsb - 1] if hsb else 0
                goal_cur = jit_goal[hsb]
                for g in range(NGRP):
                    sc = ps_sc.tile([128, 1024], F32, tag="sc", name="sc")
                    # two concurrent K=64 strip matmuls (rows 0:64 / 64:128)
                    for half in range(2):
                        tt = 2 * g + half
                        r = slice(0, D) if half == 0 else slice(D, 128)
                        nc.tensor.matmul(
                            sc[:, half * SBW : (half + 1) * SBW],
                            ktp[h][r, tt * 128 : (tt + 1) * 128],
                            qd[h][r, s0 : s0 + SBW],
                            start=True,
                            stop=True,
                        )
                    pr = pp.tile([128, 1024], F16, tag="pr", name="pr")
                    nc.scalar.activation(
                        pr[:], sc[:], mybir.ActivationFunctionType.Exp
                    )
                    run_jit(goal_prev + (goal_cur - goal_prev) * (g + 1) // NGRP)
                    last = (h, s0) if g == NGRP - 1 else None
                    pending.append((h, ctx, 2 * g, pr, last))
                    maxlag = 5 if hsb < 2 else (2 if hsb == NHL * NSB - 1 else 3)
                    flush_pending(len(pending) - maxlag)
            flush_pending(len(pending))

    nc.compile()
    return nc


def _get_nc():
    global _CACHED_NC
    if _CACHED_NC is None:
        _CACHED_NC = _build_nc()
    return _CACHED_NC


def kernel(
    hidden_states, attention_mask, Wq, bq, Wk, bk, Wv, bv
) -> np.ndarray:
    global LAST_EXEC_TIME_NS
    hidden_states = np.asarray(hidden_states, dtype=np.float32)
    attention_mask = np.asarray(attention_mask, dtype=np.float32)
    Wq = np.asarray(Wq, dtype=np.float32)
    Wk = np.asarray(Wk, dtype=np.float32)
    Wv = np.asarray(Wv, dtype=np.float32)
    bq = np.asarray(bq, dtype=np.float32)
    bk = np.asarray(bk, dtype=np.float32)
    bv = np.asarray(bv, dtype=np.float32)

    scale = 1.0 / np.sqrt(np.float32(D))

    in_maps = []
    for c in range(8):
        b, g = divmod(c, 4)
        cols = slice(g * DG, (g + 1) * DG)
        wq = Wq[:, cols] * scale
        wk = Wk[:, cols]
        wv = Wv[:, cols]
        bq_, bk_ = bq[cols] * scale, bk[cols]
        wqk = np.zeros((HID, NHL * 128), dtype=np.float32)
        bqk = np.zeros((128, NHL), dtype=np.float32)
        for h in range(NHL):
            wqk[:, h * 128 : h * 128 + D] = wq[:, h * D : (h + 1) * D]
            wqk[:, h * 128 + D : (h + 1) * 128] = wk[:, h * D : (h + 1) * D]
            bqk[0:D, h] = bq_[h * D : (h + 1) * D]
            bqk[D:128, h] = bk_[h * D : (h + 1) * D]
        # partition-major relayouts: [hid, n] -> [128, kc*n]
        wqk_pm = np.ascontiguousarray(
            wqk.reshape(KC, 128, NHL * 128).transpose(1, 0, 2).reshape(128, -1)
        )
        wv_pm = np.ascontiguousarray(
            wv.reshape(KC, 128, DG).transpose(1, 0, 2).reshape(128, -1)
        )
        maskr = np.ascontiguousarray(
            attention_mask[b, 0, 0, :].reshape(NT, 128).T
        )
        in_maps.append(
            {
                "hsT": np.ascontiguousarray(hidden_states[b].T).astype(np.float16),
                "wqk": wqk_pm.astype(np.float16),
                "wv": wv_pm.astype(np.float16),
                "bqk": bqk,
                "maskr": maskr.astype(np.float32),
            }
        )

    nc = _get_nc()
    trace = bool(os.environ.get("BASS_KERNEL_TRACE"))
    res = run_bass_kernel_spmd(nc, in_maps, list(range(8)), trace=trace)
    LAST_EXEC_TIME_NS = res.exec_time_ns

    out = np.empty((B, S, HID), dtype=np.float32)
    for c in range(8):
        b, g = divmod(c, 4)
        ctxa = res.results[c]["ctxa"]  # [3, 65, 2048]
        for hl in range(NHL):
            ctx = ctxa[hl, :D, :] / ctxa[hl, D : D + 1, :]  # [64, 2048]
            out[b, :, g * DG + hl * D : g * DG + (hl + 1) * D] = (
                ctx.T + bv[g * DG + hl * D : g * DG + (hl + 1) * D]
            )
    return out
